# revision 12
# baseline (speedup 1.0000x reference)
"""Trainium2 Bass kernel for nn_CombineGraph (GNN message passing).

Algorithm (mathematically equal to the reference):
    h0  = 0.5*(feat_a @ Wp_a + feat_b @ Wp_b);  h0[0] = 0
    per relation r:  num_r = seg_sum((w/den_r[dst]) * h0[src], dst)
                     (den_r = seg_sum(w, dst) folded into edge weights on host)
    agg = sum_r num_r @ Wr[r]                  # Wr moved past the seg_sum
    out = LN(h0 + agg) * gamma + beta;  out[0] = 0

Distribution: nodes (and their incoming edges — dst sharding) split across 8
cores.  Each core projects its own node shard to h0 (bf16), the h0 table is
AllGather-ed, then each core gathers h0[src] rows with the GPSIMD dma_gather
ucode for its dst-sorted edge tiles.  A per-tile selection matrix
W_sel[e,n] = w'_e * (dst_e == n) turns the weighted segment-sum into one
128x128x256 matmul per 128-edge tile accumulating num_r in PSUM.

dma_gather uses int16 indices, so the 100352-row table is gathered in 4
quarter slices (edges grouped by table-row quarter inside each window/rel
group; groups padded to whole 128-edge tiles with weight-0 slots).
"""
import os
import sys
import math

sys.path.insert(0, "/opt/trn_rl_repo")

import numpy as np
import ml_dtypes

P = 128
NQ = 4            # table quarters (int16 index reach)


class Cfg:
    def __init__(self, n_nodes=100000, e_per_rel=500000, n_rel=3, da=256, db=384,
                 hid=256, n_cores=8, feat_blk=4, wpb=2):
        self.n_nodes = n_nodes
        self.e_per_rel = e_per_rel
        self.n_rel = n_rel
        self.da = da
        self.db = db
        self.hid = hid
        self.n_cores = n_cores
        self.nt = math.ceil(n_nodes / n_cores / P)      # windows per core
        self.shard = self.nt * P
        self.npad = self.shard * n_cores
        assert self.npad % NQ == 0
        self.qrows = self.npad // NQ                    # rows per table quarter
        assert self.qrows <= 32768
        self.fa = da // P
        self.fb = db // P
        self.hh = hid // P
        self.feat_blk = feat_blk
        self.wpb = wpb                                  # windows per gather batch


def _prep(cfg, edge_src, edge_dst, edge_weight):
    """Group edges by (dst-core, window, rel, table-quarter of src); fold the
    per-relation denominator into the edge weights; pad groups to whole
    128-edge tiles (tile counts shared across cores for SPMD).

    Returns (T, per_core):
      T[w][r][q]  tile count (max over cores)
      per_core    dicts with gidx (int16), dstloc/ew (f32) arrays
    """
    NC, NT, NR = cfg.n_cores, cfg.nt, cfg.n_rel
    E = cfg.e_per_rel
    n = cfg.n_nodes

    def remap(src):
        k2 = src // cfg.shard
        loc = src % cfg.shard
        return k2 * cfg.shard + (loc % P) * NT + (loc // P)

    counts = np.zeros((NC, NT, NR, NQ), dtype=np.int64)
    per_rel = []
    for r in range(NR):
        s = slice(r * E, (r + 1) * E)
        dst = edge_dst[s].astype(np.int64)
        src = edge_src[s].astype(np.int64)
        w = edge_weight[s].astype(np.float64)
        den = np.bincount(dst, weights=w, minlength=n)
        wn = (w / np.maximum(den, 1e-8)[dst]).astype(np.float32)
        row = remap(src)
        q = row // cfg.qrows
        core = dst // cfg.shard
        win = (dst % cfg.shard) // P
        key = ((core * NT + win) * NR + 0) * NQ + q  # rel folded later
        key = (core * NT + win) * NQ + q
        order = np.argsort(key, kind="stable")
        dst, row, wn, q, key = dst[order], row[order], wn[order], q[order], key[order]
        counts[:, :, r, :] = np.bincount(key, minlength=NC * NT * NQ).reshape(
            NC, NT, NQ)
        per_rel.append((dst, row, wn, key))

    T = np.ceil(counts.max(axis=0) / P).astype(np.int64)      # [NT, NR, NQ]
    # column enumeration order: for w: for q: for r: T[w][r][q] tiles
    ncols_wq = T.sum(axis=1)                                  # [NT, NQ]
    ncols_w = ncols_wq.sum(axis=1)                            # [NT]
    TK = int(ncols_w.sum())

    colbase = np.zeros((NT, NQ, cfg.n_rel), dtype=np.int64)
    c = 0
    for w in range(NT):
        for q in range(NQ):
            for r in range(cfg.n_rel):
                colbase[w, q, r] = c
                c += int(T[w, r, q])
    assert c == TK

    per_core = []
    for k in range(NC):
        gidx = np.zeros((P, TK), dtype=np.int16)
        dstloc = np.zeros((P, TK), dtype=np.float32)
        ew = np.zeros((P, TK), dtype=np.float32)
        for r in range(cfg.n_rel):
            dst, row, wn, key = per_rel[r]
            lo = np.searchsorted(key, (k * NT) * NQ)
            hi = np.searchsorted(key, ((k + 1) * NT) * NQ)
            dk, rk, wk, keyk = dst[lo:hi], row[lo:hi], wn[lo:hi], key[lo:hi]
            gkey = keyk - (k * NT) * NQ          # win*NQ + q
            cnt = counts[k, :, r, :].reshape(-1)  # [NT*NQ]
            starts = np.zeros(NT * NQ, dtype=np.int64)
            starts[1:] = np.cumsum(cnt)[:-1]
            j = np.arange(len(dk)) - starts[gkey]
            win = gkey // NQ
            q = gkey % NQ
            col = colbase[win, q, r] + j // P
            part = j % P
            gidx[part, col] = (rk - q * cfg.qrows).astype(np.int16)
            dstloc[part, col] = (dk % P).astype(np.float32)
            ew[part, col] = wk
        per_core.append({"gidx": gidx, "dstloc": dstloc, "ew": ew})
    return T, colbase, TK, per_core


def _gather_plan(cfg, T):
    """Batches of wpb windows; per batch one gather per quarter.

    Returns list of batches; each batch:
      {'windows': [w..], 'gath': [(q, col_lo, n_tiles)], }
    with col ranges in the global column enumeration (w-major, q, r).
    """
    NT, NR = cfg.nt, cfg.n_rel
    batches = []
    w0 = 0
    while w0 < NT:
        ws = list(range(w0, min(w0 + cfg.wpb, NT)))
        batches.append(ws)
        w0 += cfg.wpb
    return batches


def _build(cfg, T, colbase, TK, apply_affine):
    import concourse.bass as bass
    import concourse.bacc as bacc
    import concourse.tile as tile
    from concourse import mybir

    NC, NT, NR = cfg.n_cores, cfg.nt, cfg.n_rel
    HID = cfg.hid
    FA, FB, HH = cfg.fa, cfg.fb, cfg.hh
    bf16 = mybir.dt.bfloat16
    f32 = mybir.dt.float32
    i16 = mybir.dt.int16
    Alu = mybir.AluOpType
    Act = mybir.ActivationFunctionType

    # idx DRAM layout: [128, TK*8] int16 — per tile-column 128 idxs wrapped in
    # 16 partitions (8 int16 per partition) and replicated across the 8
    # Q7-core partition groups.  Column c occupies idx columns [c*8,(c+1)*8).
    nc = bacc.Bacc("TRN2", num_devices=NC, debug=False,
                   target_bir_lowering=False)

    featta = nc.dram_tensor("featta", [P, FA, cfg.shard], bf16, kind="ExternalInput")
    feattb = nc.dram_tensor("feattb", [P, FB, cfg.shard], bf16, kind="ExternalInput")
    wpa = nc.dram_tensor("wpa", [P, FA, HID], bf16, kind="ExternalInput")
    wpb = nc.dram_tensor("wpb", [P, FB, HID], bf16, kind="ExternalInput")
    wrt = nc.dram_tensor("wrt", [P, NR, HH, HID], bf16, kind="ExternalInput")
    gidx_d = nc.dram_tensor("gidx", [P, max(TK, 1) * 8], i16, kind="ExternalInput")
    dstloc_d = nc.dram_tensor("dstloc", [P, max(TK, 1)], f32, kind="ExternalInput")
    ew_d = nc.dram_tensor("ew", [P, max(TK, 1)], f32, kind="ExternalInput")
    mask_d = nc.dram_tensor("mask", [1, 1], f32, kind="ExternalInput")
    if apply_affine:
        gamma_d = nc.dram_tensor("gamma", [P, HID], f32, kind="ExternalInput")
        beta_d = nc.dram_tensor("beta", [P, HID], f32, kind="ExternalInput")
    h1 = nc.dram_tensor("h1", [cfg.shard, HID], f32, kind="ExternalOutput")

    h0_shard = nc.dram_tensor("h0_shard", [P, NT, HID], bf16, kind="Internal")
    h0_table = nc.dram_tensor("h0_table", [cfg.npad, HID], bf16, kind="Internal",
                              addr_space="Shared")

    batches = _gather_plan(cfg, T)

    with tile.TileContext(nc) as tc:
        import contextlib
        ctx = contextlib.ExitStack()
        with ctx:
            singles = ctx.enter_context(tc.tile_pool(name="singles", bufs=1))
            featp = ctx.enter_context(tc.tile_pool(name="featp", bufs=2))
            psA = ctx.enter_context(tc.tile_pool(name="psA", bufs=2, space="PSUM"))
            idxp = ctx.enter_context(tc.tile_pool(name="idxp", bufs=3))
            landp = ctx.enter_context(tc.tile_pool(name="landp", bufs=10))
            wselp = ctx.enter_context(tc.tile_pool(name="wselp", bufs=8))
            nump = ctx.enter_context(tc.tile_pool(name="nump", bufs=4, space="PSUM"))
            aggp = ctx.enter_context(tc.tile_pool(name="aggp", bufs=2, space="PSUM"))
            xp = ctx.enter_context(tc.tile_pool(name="xp", bufs=3))
            xtp = ctx.enter_context(tc.tile_pool(name="xtp", bufs=4))
            yp = ctx.enter_context(tc.tile_pool(name="yp", bufs=2))
            otp = ctx.enter_context(tc.tile_pool(name="otp", bufs=2))
            smallp = ctx.enter_context(tc.tile_pool(name="smallp", bufs=8))

            # ---- constants ----
            wpa_sb = singles.tile([P, FA, HID], bf16)
            nc.sync.dma_start(out=wpa_sb[:], in_=wpa[:, :, :])
            wpb_sb = singles.tile([P, FB, HID], bf16)
            nc.sync.dma_start(out=wpb_sb[:], in_=wpb[:, :, :])
            wr_sb = singles.tile([P, NR, HH, HID], bf16)
            nc.sync.dma_start(out=wr_sb[:], in_=wrt[:, :, :, :])
            mask_sb = singles.tile([1, 1], f32)
            nc.sync.dma_start(out=mask_sb[:], in_=mask_d[:, :])
            iota_sb = singles.tile([P, P], bf16)
            nc.gpsimd.iota(iota_sb[:], pattern=[[1, P]], base=0,
                           channel_multiplier=0,
                           allow_small_or_imprecise_dtypes=True)
            eps_sb = singles.tile([P, 1], f32)
            nc.vector.memset(eps_sb[:], 1e-5)
            if apply_affine:
                gamma_sb = singles.tile([P, HID], f32)
                nc.sync.dma_start(out=gamma_sb[:], in_=gamma_d[:, :])
                beta_sb = singles.tile([P, HID], f32)
                nc.sync.dma_start(out=beta_sb[:], in_=beta_d[:, :])

            dst_sb = singles.tile([P, max(TK, 1)], f32)
            nc.sync.dma_start(out=dst_sb[:], in_=dstloc_d[:, :])
            ew_sb = singles.tile([P, max(TK, 1)], f32)
            nc.sync.dma_start(out=ew_sb[:], in_=ew_d[:, :])

            # ---- phase A: h0 projection into persistent SBUF table ----
            h0_sb = singles.tile([P, NT, HID], bf16)
            BLK = cfg.feat_blk
            nblk = (NT + BLK - 1) // BLK
            for b in range(nblk):
                j0 = b * BLK
                j1 = min(NT, j0 + BLK)
                fa_t = featp.tile([P, FA, (j1 - j0) * P], bf16, tag="fa")
                nc.sync.dma_start(out=fa_t[:], in_=featta[:, :, j0 * P:j1 * P])
                fb_t = featp.tile([P, FB, (j1 - j0) * P], bf16, tag="fb")
                nc.sync.dma_start(out=fb_t[:], in_=feattb[:, :, j0 * P:j1 * P])
                for j in range(j1 - j0):
                    nt_i = j0 + j
                    ps = psA.tile([P, HID], f32)
                    for c in range(FA):
                        nc.tensor.matmul(ps[:], lhsT=fa_t[:, c, j * P:(j + 1) * P],
                                         rhs=wpa_sb[:, c, :],
                                         start=(c == 0), stop=False)
                    for c in range(FB):
                        nc.tensor.matmul(ps[:], lhsT=fb_t[:, c, j * P:(j + 1) * P],
                                         rhs=wpb_sb[:, c, :],
                                         start=False, stop=(c == FB - 1))
                    nc.scalar.activation(out=h0_sb[:, nt_i, :], in_=ps[:],
                                         func=Act.Copy, scale=0.5)
                    if nt_i == 0:
                        nc.vector.tensor_scalar(
                            out=h0_sb[0:1, 0, :], in0=h0_sb[0:1, 0, :],
                            scalar1=mask_sb[0:1, 0:1], scalar2=None, op0=Alu.mult)

            nc.sync.dma_start(out=h0_shard[:, :, :], in_=h0_sb[:])
            nc.gpsimd.collective_compute(
                "AllGather", mybir.AluOpType.bypass,
                replica_groups=[list(range(NC))],
                ins=[h0_shard[:, :, :]],
                outs=[h0_table[:, :]],
            )

            # ---- phase B ----
            for ws in batches:
                # per quarter: one gather covering all (w in ws, r) tiles
                lands = {}
                for q in range(NQ):
                    ntile_q = int(sum(T[w, r, q] for w in ws for r in range(NR)))
                    if ntile_q == 0:
                        continue
                    col_lo = int(colbase[ws[0], q, 0])
                    # columns for this (batch, q) are contiguous only if wpb==1;
                    # otherwise gather per (w, q) chunks within the batch.
                    for w in ws:
                        nt_w = int(T[w, :, q].sum())
                        if nt_w == 0:
                            continue
                        cl = int(colbase[w, q, 0])
                        land = landp.tile([P, nt_w, HID], bf16, tag="land")
                        idx_sb = idxp.tile([P, nt_w * 8], i16, tag="idx")
                        nc.sync.dma_start(out=idx_sb[:],
                                          in_=gidx_d[:, cl * 8:(cl + nt_w) * 8])
                        nidx = nt_w * P
                        nc.gpsimd.dma_gather(
                            land[:, :, :],
                            h0_table[q * cfg.qrows:(q + 1) * cfg.qrows, :],
                            idx_sb[:],
                            nidx, nidx, HID,
                        )
                        lands[(w, q)] = (land, cl)

                for w in ws:
                    kw = int(T[w].sum())
                    have_edges = kw > 0
                    if have_edges:
                        agg = aggp.tile([P, HID], f32)
                        active = [r for r in range(NR) if T[w, r, :].sum() > 0]
                        for r in range(NR):
                            twr = int(T[w, r, :].sum())
                            if twr == 0:
                                continue
                            num = nump.tile([P, HID], f32, tag="num")
                            tdone = 0
                            for q in range(NQ):
                                tq = int(T[w, r, q])
                                if tq == 0:
                                    continue
                                land, cl = lands[(w, q)]
                                for t in range(tq):
                                    col = int(colbase[w, q, r]) + t
                                    lcol = col - cl
                                    wsel = wselp.tile([P, P], bf16, tag="wsel")
                                    nc.vector.tensor_scalar(
                                        out=wsel[:], in0=iota_sb[:],
                                        scalar1=dst_sb[:, col:col + 1],
                                        scalar2=ew_sb[:, col:col + 1],
                                        op0=Alu.is_equal, op1=Alu.mult)
                                    nc.tensor.matmul(
                                        num[:], lhsT=wsel[:],
                                        rhs=land[:, lcol, :],
                                        start=(tdone == 0),
                                        stop=(tdone == twr - 1))
                                    tdone += 1
                            xr = xp.tile([P, HID], bf16, tag="xr")
                            nc.scalar.activation(out=xr[:], in_=num[:],
                                                 func=Act.Copy)
                            for h in range(HH):
                                xt = xtp.tile([P, P], bf16, tag="xt")
                                nc.sync.dma_start(out=xt[:],
                                                  in_=xr[:, h * P:(h + 1) * P],
                                                  transpose=True)
                                nc.tensor.matmul(
                                    agg[:], lhsT=xt[:], rhs=wr_sb[:, r, h, :],
                                    start=(r == active[0] and h == 0),
                                    stop=(r == active[-1] and h == HH - 1))

                    y = yp.tile([P, HID], f32, tag="y")
                    if have_edges:
                        nc.vector.tensor_tensor(out=y[:], in0=agg[:],
                                                in1=h0_sb[:, w, :], op=Alu.add)
                    else:
                        nc.vector.tensor_copy(out=y[:], in_=h0_sb[:, w, :])

                    stats = smallp.tile([P, 6], f32, tag="stats")
                    nc.vector.bn_stats(out=stats[:], in_=y[:])
                    mv = smallp.tile([P, 2], f32, tag="mv")
                    nc.vector.bn_aggr(out=mv[:], in_=stats[:])
                    rstd = smallp.tile([P, 1], f32, tag="rstd")
                    nc.scalar.activation(out=rstd[:], in_=mv[:, 1:2],
                                         func=Act.Sqrt, bias=eps_sb[:, 0:1])
                    nc.vector.reciprocal(out=rstd[:], in_=rstd[:])
                    ot = otp.tile([P, HID], f32, tag="ot")
                    nc.vector.tensor_scalar(out=ot[:], in0=y[:],
                                            scalar1=mv[:, 0:1],
                                            scalar2=rstd[:, 0:1],
                                            op0=Alu.subtract, op1=Alu.mult)
                    if apply_affine:
                        nc.vector.tensor_tensor(out=ot[:], in0=ot[:],
                                                in1=gamma_sb[:], op=Alu.mult)
                        nc.vector.tensor_tensor(out=ot[:], in0=ot[:],
                                                in1=beta_sb[:], op=Alu.add)
                    if w == 0:
                        nc.vector.tensor_scalar(out=ot[0:1, :], in0=ot[0:1, :],
                                                scalar1=mask_sb[0:1, 0:1],
                                                scalar2=None, op0=Alu.mult)
                    nc.sync.dma_start(out=h1[w * P:(w + 1) * P, :], in_=ot[:])
    nc.compile()
    return nc


def _expand_idx_v2(gidx):
    """[P, TK] -> [P, TK*8] int16 in dma_gather layout."""
    Pp, TK = gidx.shape
    wrapped = np.zeros((16, TK, 8), dtype=np.int16)
    pos = np.arange(P)
    wrapped[pos % 16, :, pos // 16] = gidx[pos, :]
    block = wrapped.transpose(0, 1, 2).reshape(16, TK * 8)
    return np.tile(block, (8, 1))


def _make_in_maps(cfg, inputs, per_core, apply_affine):
    feat_a = np.asarray(inputs["feat_a"], dtype=np.float32)
    feat_b = np.asarray(inputs["feat_b"], dtype=np.float32)
    n = feat_a.shape[0]
    fa_pad = np.zeros((cfg.npad, cfg.da), dtype=ml_dtypes.bfloat16)
    fa_pad[:n] = feat_a.astype(ml_dtypes.bfloat16)
    fb_pad = np.zeros((cfg.npad, cfg.db), dtype=ml_dtypes.bfloat16)
    fb_pad[:n] = feat_b.astype(ml_dtypes.bfloat16)

    wpa = np.ascontiguousarray(
        np.asarray(inputs["Wp_a"], dtype=np.float32)
        .reshape(cfg.fa, P, cfg.hid).transpose(1, 0, 2)).astype(ml_dtypes.bfloat16)
    wpb = np.ascontiguousarray(
        np.asarray(inputs["Wp_b"], dtype=np.float32)
        .reshape(cfg.fb, P, cfg.hid).transpose(1, 0, 2)).astype(ml_dtypes.bfloat16)
    wr = np.ascontiguousarray(
        np.asarray(inputs["Wr"], dtype=np.float32)
        .reshape(cfg.n_rel, cfg.hh, P, cfg.hid).transpose(2, 0, 1, 3)
    ).astype(ml_dtypes.bfloat16)

    in_maps = []
    for k in range(cfg.n_cores):
        sl = slice(k * cfg.shard, (k + 1) * cfg.shard)
        fta = np.ascontiguousarray(
            fa_pad[sl].T.reshape(cfg.fa, P, cfg.shard).transpose(1, 0, 2))
        ftb = np.ascontiguousarray(
            fb_pad[sl].T.reshape(cfg.fb, P, cfg.shard).transpose(1, 0, 2))
        m = {
            "featta": fta, "feattb": ftb,
            "wpa": wpa, "wpb": wpb, "wrt": wr,
            "gidx": _expand_idx_v2(per_core[k]["gidx"]),
            "dstloc": per_core[k]["dstloc"],
            "ew": per_core[k]["ew"],
            "mask": np.array([[0.0 if k == 0 else 1.0]], dtype=np.float32),
        }
        if apply_affine:
            m["gamma"] = np.broadcast_to(
                np.asarray(inputs["ln_gamma"], np.float32), (P, cfg.hid)).copy()
            m["beta"] = np.broadcast_to(
                np.asarray(inputs["ln_beta"], np.float32), (P, cfg.hid)).copy()
        in_maps.append(m)
    return in_maps


def run(cfg, inputs, sim=False):
    edge_src = np.asarray(inputs["edge_src"])
    edge_dst = np.asarray(inputs["edge_dst"])
    edge_weight = np.asarray(inputs["edge_weight"], dtype=np.float32)
    gamma = np.asarray(inputs["ln_gamma"], dtype=np.float32)
    beta = np.asarray(inputs["ln_beta"], dtype=np.float32)
    apply_affine = not (np.all(gamma == 1.0) and np.all(beta == 0.0))

    T, colbase, TK, per_core = _prep(cfg, edge_src, edge_dst, edge_weight)
    nc = _build(cfg, T, colbase, TK, apply_affine)
    in_maps = _make_in_maps(cfg, inputs, per_core, apply_affine)

    if sim:
        from concourse.bass_interp import MultiCoreSim
        msim = MultiCoreSim(nc, num_cores=cfg.n_cores)
        for k, core in enumerate(msim.cores.values()):
            for name, arr in in_maps[k].items():
                core.tensor(name)[:] = arr
        msim.simulate()
        results = [{"h1": np.array(core.tensor("h1"))}
                   for core in msim.cores.values()]
        exec_ns = None
    else:
        from concourse import bass_utils
        trace = bool(os.environ.get("BASS_KERNEL_TRACE"))
        if trace:
            _install_ntff_shim()
            bass_utils.upload_artifacts = lambda tmpdir: f"local://{tmpdir}"
        try:
            res = bass_utils.run_bass_kernel_spmd(
                nc, in_maps, core_ids=list(range(cfg.n_cores)), trace=trace)
        except Exception:
            if not trace:
                raise
            res = bass_utils.run_bass_kernel_spmd(
                nc, in_maps, core_ids=list(range(cfg.n_cores)), trace=False)
        results = res.results
        exec_ns = res.exec_time_ns
        if exec_ns is not None:
            print(f"HW exec time: {exec_ns} ns", flush=True)

    out = np.concatenate([r["h1"] for r in results], axis=0)[:cfg.n_nodes]
    return np.ascontiguousarray(out.astype(np.float32)), exec_ns


def _install_ntff_shim():
    """The image's antenv lacks axon_hooks; register the NTFF profile hook
    (ctypes over libaxon_pjrt.so) as a synthetic module so bass_utils'
    trace path finds it."""
    import types
    import ctypes
    import contextlib

    name = "antenv.axon_hooks"
    if name in sys.modules:
        return
    try:
        import antenv.axon_hooks  # noqa: F401
        return
    except ImportError:
        pass
    mod = types.ModuleType(name)
    state = {"hook": None}
    mod.set_axon_ntff_profile_hook = lambda h: state.__setitem__("hook", h)
    mod.get_axon_ntff_profile_hook = lambda: state["hook"]
    so = os.environ.get("AXON_PJRT_SO", "/opt/axon/libaxon_pjrt.so")
    if os.path.exists(so):
        try:
            lib = ctypes.CDLL(so)
            if hasattr(lib, "axon_start_nrt_profile"):
                lib.axon_start_nrt_profile.argtypes = [
                    ctypes.POINTER(ctypes.c_int64), ctypes.c_size_t]
                lib.axon_start_nrt_profile.restype = ctypes.c_int64
                lib.axon_stop_nrt_profile.argtypes = [ctypes.c_char_p]
                lib.axon_stop_nrt_profile.restype = ctypes.c_int64

                @contextlib.contextmanager
                def _hook(output_dir, device_ids):
                    import jax
                    jax.devices()
                    if device_ids:
                        ids = (ctypes.c_int64 * len(device_ids))(*device_ids)
                        rc = lib.axon_start_nrt_profile(ids, len(device_ids))
                    else:
                        rc = lib.axon_start_nrt_profile(None, 0)
                    if rc != 0:
                        raise RuntimeError(f"axon_start_nrt_profile rc={rc}")
                    try:
                        yield
                    finally:
                        nfiles = lib.axon_stop_nrt_profile(
                            str(output_dir).encode())
                        print(f"profile: {nfiles} file(s) -> {output_dir}",
                              flush=True)

                state["hook"] = _hook
        except Exception:
            pass
    sys.modules[name] = mod


def kernel(**inputs):
    cfg = Cfg()
    out, _ = run(cfg, inputs, sim=False)
    return out


# revision 19
# speedup vs baseline: 1.6083x; 1.6083x over previous
"""Trainium2 Bass kernel for nn_CombineGraph (GNN message passing).

Algorithm (mathematically equal to the reference):
    h0  = 0.5*(feat_a @ Wp_a + feat_b @ Wp_b);  h0[0] = 0
    per relation r:  num_r = seg_sum((w/den_r[dst]) * h0[src], dst)
                     (den_r = seg_sum(w, dst) folded into edge weights on host)
    agg = sum_r num_r @ Wr[r]                  # Wr moved past the seg_sum
    out = LN(h0 + agg) * gamma + beta;  out[0] = 0

Distribution: nodes (and their incoming edges — dst sharding) split across 8
cores.  Each core projects its own node shard to h0 (bf16), the h0 table is
AllGather-ed, then each core gathers h0[src] rows with the GPSIMD dma_gather
ucode for its dst-sorted edge tiles.  A per-tile selection matrix
W_sel[e,n] = w'_e * (dst_e == n) turns the weighted segment-sum into one
128x128x256 matmul per 128-edge tile accumulating num_r in PSUM.

dma_gather uses int16 indices, so the 100352-row table is gathered in 4
quarter slices (edges grouped by table-row quarter inside each window/rel
group; groups padded to whole 128-edge tiles with weight-0 slots).
"""
import os
import sys
import math

sys.path.insert(0, "/opt/trn_rl_repo")

import numpy as np
import ml_dtypes

P = 128
NQ = 4            # table quarters (int16 index reach)


class Cfg:
    def __init__(self, n_nodes=100000, e_per_rel=500000, n_rel=3, da=256, db=384,
                 hid=256, n_cores=8, feat_blk=4, wpb=2):
        self.n_nodes = n_nodes
        self.e_per_rel = e_per_rel
        self.n_rel = n_rel
        self.da = da
        self.db = db
        self.hid = hid
        self.n_cores = n_cores
        self.nt = math.ceil(n_nodes / n_cores / P)      # windows per core
        self.shard = self.nt * P
        self.npad = self.shard * n_cores
        assert self.npad % NQ == 0
        self.qrows = self.npad // NQ                    # rows per table quarter
        assert self.qrows <= 32768
        self.fa = da // P
        self.fb = db // P
        self.hh = hid // P
        self.feat_blk = feat_blk
        self.wpb = wpb                                  # windows per gather batch


def _prep(cfg, edge_src, edge_dst, edge_weight):
    """Group edges by (dst-core, window, rel, table-quarter of src); fold the
    per-relation denominator into the edge weights; pad groups to whole
    128-edge tiles (tile counts shared across cores for SPMD).

    Returns (T, per_core):
      T[w][r][q]  tile count (max over cores)
      per_core    dicts with gidx (int16), dstloc/ew (f32) arrays
    """
    NC, NT, NR = cfg.n_cores, cfg.nt, cfg.n_rel
    E = cfg.e_per_rel
    n = cfg.n_nodes

    def remap(src):
        k2 = src // cfg.shard
        loc = src % cfg.shard
        return k2 * cfg.shard + (loc % P) * NT + (loc // P)

    counts = np.zeros((NC, NT, NR, NQ), dtype=np.int64)
    per_rel = []
    for r in range(NR):
        s = slice(r * E, (r + 1) * E)
        dst = edge_dst[s].astype(np.int64)
        src = edge_src[s].astype(np.int64)
        w = edge_weight[s].astype(np.float64)
        den = np.bincount(dst, weights=w, minlength=n)
        wn = (w / np.maximum(den, 1e-8)[dst]).astype(np.float32)
        row = remap(src)
        q = row // cfg.qrows
        core = dst // cfg.shard
        win = (dst % cfg.shard) // P
        key = ((core * NT + win) * NR + 0) * NQ + q  # rel folded later
        key = (core * NT + win) * NQ + q
        order = np.argsort(key, kind="stable")
        dst, row, wn, q, key = dst[order], row[order], wn[order], q[order], key[order]
        counts[:, :, r, :] = np.bincount(key, minlength=NC * NT * NQ).reshape(
            NC, NT, NQ)
        per_rel.append((dst, row, wn, key))

    T = np.ceil(counts.max(axis=0) / P).astype(np.int64)      # [NT, NR, NQ]
    # column enumeration order: for batch: for q: for w in batch: for r
    TK = int(T.sum())
    colbase = np.zeros((NT, NQ, cfg.n_rel), dtype=np.int64)
    c = 0
    w0 = 0
    while w0 < NT:
        ws = range(w0, min(w0 + cfg.wpb, NT))
        for q in range(NQ):
            for w in ws:
                for r in range(cfg.n_rel):
                    colbase[w, q, r] = c
                    c += int(T[w, r, q])
        w0 += cfg.wpb
    assert c == TK

    per_core = []
    for k in range(NC):
        gidx = np.zeros((P, TK), dtype=np.int16)
        dstloc = np.zeros((P, TK), dtype=np.float32)
        ew = np.zeros((P, TK), dtype=np.float32)
        for r in range(cfg.n_rel):
            dst, row, wn, key = per_rel[r]
            lo = np.searchsorted(key, (k * NT) * NQ)
            hi = np.searchsorted(key, ((k + 1) * NT) * NQ)
            dk, rk, wk, keyk = dst[lo:hi], row[lo:hi], wn[lo:hi], key[lo:hi]
            gkey = keyk - (k * NT) * NQ          # win*NQ + q
            cnt = counts[k, :, r, :].reshape(-1)  # [NT*NQ]
            starts = np.zeros(NT * NQ, dtype=np.int64)
            starts[1:] = np.cumsum(cnt)[:-1]
            j = np.arange(len(dk)) - starts[gkey]
            win = gkey // NQ
            q = gkey % NQ
            col = colbase[win, q, r] + j // P
            part = j % P
            gidx[part, col] = (rk - q * cfg.qrows).astype(np.int16)
            dstloc[part, col] = (dk % P).astype(np.float32)
            ew[part, col] = wk
        per_core.append({"gidx": gidx, "dstloc": dstloc, "ew": ew})
    return T, colbase, TK, per_core


def _gather_plan(cfg, T):
    """Batches of wpb windows; per batch one gather per quarter.

    Returns list of batches; each batch:
      {'windows': [w..], 'gath': [(q, col_lo, n_tiles)], }
    with col ranges in the global column enumeration (w-major, q, r).
    """
    NT, NR = cfg.nt, cfg.n_rel
    batches = []
    w0 = 0
    while w0 < NT:
        ws = list(range(w0, min(w0 + cfg.wpb, NT)))
        batches.append(ws)
        w0 += cfg.wpb
    return batches


def _build(cfg, T, colbase, TK, apply_affine):
    import concourse.bass as bass
    import concourse.bacc as bacc
    import concourse.tile as tile
    from concourse import mybir

    NC, NT, NR = cfg.n_cores, cfg.nt, cfg.n_rel
    HID = cfg.hid
    FA, FB, HH = cfg.fa, cfg.fb, cfg.hh
    bf16 = mybir.dt.bfloat16
    f32 = mybir.dt.float32
    i16 = mybir.dt.int16
    Alu = mybir.AluOpType
    Act = mybir.ActivationFunctionType

    # idx DRAM layout: [128, TK*8] int16 — per tile-column 128 idxs wrapped in
    # 16 partitions (8 int16 per partition) and replicated across the 8
    # Q7-core partition groups.  Column c occupies idx columns [c*8,(c+1)*8).
    nc = bacc.Bacc("TRN2", num_devices=NC, debug=False,
                   target_bir_lowering=False, dynamic_dma_scratch_size=16384)

    featta = nc.dram_tensor("featta", [P, FA, cfg.shard], bf16, kind="ExternalInput")
    feattb = nc.dram_tensor("feattb", [P, FB, cfg.shard], bf16, kind="ExternalInput")
    wpa = nc.dram_tensor("wpa", [P, FA, HID], bf16, kind="ExternalInput")
    wpb = nc.dram_tensor("wpb", [P, FB, HID], bf16, kind="ExternalInput")
    wrt = nc.dram_tensor("wrt", [P, NR, HH, HID], bf16, kind="ExternalInput")
    gidx_d = nc.dram_tensor("gidx", [P, max(TK, 1) * 8], i16, kind="ExternalInput")
    dstloc_d = nc.dram_tensor("dstloc", [P, max(TK, 1)], f32, kind="ExternalInput")
    ew_d = nc.dram_tensor("ew", [P, max(TK, 1)], f32, kind="ExternalInput")
    mask_d = nc.dram_tensor("mask", [1, 1], f32, kind="ExternalInput")
    if apply_affine:
        gamma_d = nc.dram_tensor("gamma", [P, HID], f32, kind="ExternalInput")
        beta_d = nc.dram_tensor("beta", [P, HID], f32, kind="ExternalInput")
    h1 = nc.dram_tensor("h1", [cfg.shard, HID], f32, kind="ExternalOutput")

    h0_shard = nc.dram_tensor("h0_shard", [P, NT, HID], bf16, kind="Internal")
    h0_table = nc.dram_tensor("h0_table", [cfg.npad, HID], bf16, kind="Internal",
                              addr_space="Shared")

    batches = _gather_plan(cfg, T)

    with tile.TileContext(nc) as tc:
        import contextlib
        ctx = contextlib.ExitStack()
        with ctx:
            singles = ctx.enter_context(tc.tile_pool(name="singles", bufs=1))
            featp = ctx.enter_context(tc.tile_pool(name="featp", bufs=2))
            psA = ctx.enter_context(tc.tile_pool(name="psA", bufs=2, space="PSUM"))
            idxp = ctx.enter_context(tc.tile_pool(name="idxp", bufs=2))
            metp = ctx.enter_context(tc.tile_pool(name="metp", bufs=3))
            landp = ctx.enter_context(tc.tile_pool(name="landp", bufs=6))
            wselp = ctx.enter_context(tc.tile_pool(name="wselp", bufs=16))
            nump = ctx.enter_context(tc.tile_pool(name="nump", bufs=4, space="PSUM"))
            aggp = ctx.enter_context(tc.tile_pool(name="aggp", bufs=2, space="PSUM"))
            xp = ctx.enter_context(tc.tile_pool(name="xp", bufs=4))
            yp = ctx.enter_context(tc.tile_pool(name="yp", bufs=2))
            otp = ctx.enter_context(tc.tile_pool(name="otp", bufs=2))
            smallp = ctx.enter_context(tc.tile_pool(name="smallp", bufs=8))

            # ---- constants ----
            wpa_sb = singles.tile([P, FA, HID], bf16)
            nc.sync.dma_start(out=wpa_sb[:], in_=wpa[:, :, :])
            wpb_sb = singles.tile([P, FB, HID], bf16)
            nc.sync.dma_start(out=wpb_sb[:], in_=wpb[:, :, :])
            wr_sb = singles.tile([P, NR, HH, HID], bf16)
            nc.sync.dma_start(out=wr_sb[:], in_=wrt[:, :, :, :])
            mask_sb = singles.tile([1, 1], f32)
            nc.sync.dma_start(out=mask_sb[:], in_=mask_d[:, :])
            iota_sb = singles.tile([P, P], bf16)
            nc.gpsimd.iota(iota_sb[:], pattern=[[1, P]], base=0,
                           channel_multiplier=0,
                           allow_small_or_imprecise_dtypes=True)
            eps_sb = singles.tile([P, 1], f32)
            nc.vector.memset(eps_sb[:], 1e-5)
            if apply_affine:
                gamma_sb = singles.tile([P, HID], f32)
                nc.sync.dma_start(out=gamma_sb[:], in_=gamma_d[:, :])
                beta_sb = singles.tile([P, HID], f32)
                nc.sync.dma_start(out=beta_sb[:], in_=beta_d[:, :])

            # ---- phase A: h0 projection into persistent SBUF table ----
            h0_sb = singles.tile([P, NT, HID], bf16)
            BLK = cfg.feat_blk
            nblk = (NT + BLK - 1) // BLK
            for b in range(nblk):
                j0 = b * BLK
                j1 = min(NT, j0 + BLK)
                fa_t = featp.tile([P, FA, (j1 - j0) * P], bf16, tag="fa")
                nc.sync.dma_start(out=fa_t[:], in_=featta[:, :, j0 * P:j1 * P])
                fb_t = featp.tile([P, FB, (j1 - j0) * P], bf16, tag="fb")
                nc.sync.dma_start(out=fb_t[:], in_=feattb[:, :, j0 * P:j1 * P])
                for j in range(j1 - j0):
                    nt_i = j0 + j
                    ps = psA.tile([P, HID], f32)
                    for c in range(FA):
                        nc.tensor.matmul(ps[:], lhsT=fa_t[:, c, j * P:(j + 1) * P],
                                         rhs=wpa_sb[:, c, :],
                                         start=(c == 0), stop=False)
                    for c in range(FB):
                        nc.tensor.matmul(ps[:], lhsT=fb_t[:, c, j * P:(j + 1) * P],
                                         rhs=wpb_sb[:, c, :],
                                         start=False, stop=(c == FB - 1))
                    nc.scalar.activation(out=h0_sb[:, nt_i, :], in_=ps[:],
                                         func=Act.Copy, scale=0.5)
                    if nt_i == 0:
                        nc.vector.tensor_scalar(
                            out=h0_sb[0:1, 0, :], in0=h0_sb[0:1, 0, :],
                            scalar1=mask_sb[0:1, 0:1], scalar2=None, op0=Alu.mult)

            nc.sync.dma_start(out=h0_shard[:, :, :], in_=h0_sb[:])
            nc.gpsimd.collective_compute(
                "AllGather", mybir.AluOpType.bypass,
                replica_groups=[list(range(NC))],
                ins=[h0_shard[:, :, :]],
                outs=[h0_table[:, :]],
            )

            # ---- phase B ----
            for ws in batches:
                cb_lo = int(colbase[ws[0], 0, 0])
                cb_hi = cb_lo + int(T[ws[0]:ws[-1] + 1].sum())
                nb = cb_hi - cb_lo
                if nb == 0:
                    lands = {}
                else:
                    idx_sb = idxp.tile([P, nb * 8], i16, tag="idx")
                    nc.sync.dma_start(out=idx_sb[:],
                                      in_=gidx_d[:, cb_lo * 8:cb_hi * 8])
                    dst_sb = metp.tile([P, nb], f32, tag="dst")
                    nc.sync.dma_start(out=dst_sb[:],
                                      in_=dstloc_d[:, cb_lo:cb_hi])
                    ew_sb = metp.tile([P, nb], f32, tag="ew")
                    nc.sync.dma_start(out=ew_sb[:], in_=ew_d[:, cb_lo:cb_hi])
                    lands = {}
                    for q in range(NQ):
                        ntq = int(sum(T[w, r, q] for w in ws for r in range(NR)))
                        if ntq == 0:
                            continue
                        cq_lo = int(colbase[ws[0], q, 0])
                        land = landp.tile([P, ntq, HID], bf16, tag="land")
                        nidx = ntq * P
                        nc.gpsimd.dma_gather(
                            land[:, :, :],
                            h0_table[q * cfg.qrows:(q + 1) * cfg.qrows, :],
                            idx_sb[:, (cq_lo - cb_lo) * 8:(cq_lo - cb_lo + ntq) * 8],
                            nidx, nidx, HID, single_packet=False,
                        )
                        lands[q] = (land, cq_lo)

                for w in ws:
                    kw = int(T[w].sum())
                    have_edges = kw > 0
                    if have_edges:
                        agg = aggp.tile([P, HID], f32)
                        active = [r for r in range(NR) if T[w, r, :].sum() > 0]
                        for r in range(NR):
                            twr = int(T[w, r, :].sum())
                            if twr == 0:
                                continue
                            numt = nump.tile([P, HH, P], f32, tag="num")
                            work = []
                            for q in range(NQ):
                                tq = int(T[w, r, q])
                                if tq == 0:
                                    continue
                                land, cql = lands[q]
                                for t in range(tq):
                                    col = int(colbase[w, q, r]) + t
                                    lcol = col - cql
                                    wsel = wselp.tile([P, P], bf16, tag="wsel")
                                    nc.vector.tensor_scalar(
                                        out=wsel[:], in0=iota_sb[:],
                                        scalar1=dst_sb[:, col - cb_lo:col - cb_lo + 1],
                                        scalar2=ew_sb[:, col - cb_lo:col - cb_lo + 1],
                                        op0=Alu.is_equal, op1=Alu.mult)
                                    work.append((land, lcol, wsel))
                            for h in range(HH):
                                for i, (land, lcol, wsel) in enumerate(work):
                                    nc.tensor.matmul(
                                        numt[:, h, :],
                                        lhsT=land[:, lcol, h * P:(h + 1) * P],
                                        rhs=wsel[:],
                                        start=(i == 0),
                                        stop=(i == twr - 1))
                            xrt = xp.tile([P, HH, P], bf16, tag="xr")
                            nc.scalar.activation(out=xrt[:], in_=numt[:],
                                                 func=Act.Copy)
                            for h in range(HH):
                                nc.tensor.matmul(
                                    agg[:], lhsT=xrt[:, h, :],
                                    rhs=wr_sb[:, r, h, :],
                                    start=(r == active[0] and h == 0),
                                    stop=(r == active[-1] and h == HH - 1))

                    y = yp.tile([P, HID], f32, tag="y")
                    s1 = smallp.tile([P, 1], f32, tag="s1")
                    if have_edges:
                        nc.vector.tensor_tensor(out=y[:], in0=agg[:],
                                                in1=h0_sb[:, w, :], op=Alu.add)
                    else:
                        nc.vector.tensor_copy(out=y[:], in_=h0_sb[:, w, :])
                    # LN moments on ACT: s1 = sum(y), s2 = sum(y^2)
                    ycopy = otp.tile([P, HID], f32, tag="yc")
                    nc.scalar.activation(out=ycopy[:], in_=y[:], func=Act.Copy,
                                         accum_out=s1[:, 0:1])
                    y2 = xp.tile([P, HID], f32, tag="y2")
                    s2 = smallp.tile([P, 1], f32, tag="s2")
                    nc.scalar.activation(out=y2[:], in_=y[:], func=Act.Square,
                                         accum_out=s2[:, 0:1])
                    mean = smallp.tile([P, 1], f32, tag="mean")
                    nc.vector.tensor_scalar(out=mean[:], in0=s1[:],
                                            scalar1=1.0 / HID, scalar2=None,
                                            op0=Alu.mult)
                    # var = s2/H - mean^2  (computed as (s2/H) - mean*mean)
                    msq = smallp.tile([P, 1], f32, tag="msq")
                    nc.vector.tensor_scalar(out=msq[:], in0=mean[:],
                                            scalar1=mean[:, 0:1], scalar2=None,
                                            op0=Alu.mult)
                    var = smallp.tile([P, 1], f32, tag="var")
                    nc.vector.tensor_scalar(out=var[:], in0=s2[:],
                                            scalar1=1.0 / HID,
                                            scalar2=msq[:, 0:1],
                                            op0=Alu.mult, op1=Alu.subtract)
                    rstd = smallp.tile([P, 1], f32, tag="rstd")
                    nc.scalar.activation(out=rstd[:], in_=var[:],
                                         func=Act.Sqrt, bias=eps_sb[:, 0:1])
                    nc.vector.reciprocal(out=rstd[:], in_=rstd[:])
                    ot = otp.tile([P, HID], f32, tag="ot")
                    nc.vector.tensor_scalar(out=ot[:], in0=y[:],
                                            scalar1=mean[:, 0:1],
                                            scalar2=rstd[:, 0:1],
                                            op0=Alu.subtract, op1=Alu.mult)
                    if apply_affine:
                        nc.vector.tensor_tensor(out=ot[:], in0=ot[:],
                                                in1=gamma_sb[:], op=Alu.mult)
                        nc.vector.tensor_tensor(out=ot[:], in0=ot[:],
                                                in1=beta_sb[:], op=Alu.add)
                    if w == 0:
                        nc.vector.tensor_scalar(out=ot[0:1, :], in0=ot[0:1, :],
                                                scalar1=mask_sb[0:1, 0:1],
                                                scalar2=None, op0=Alu.mult)
                    nc.sync.dma_start(out=h1[w * P:(w + 1) * P, :], in_=ot[:])
    nc.compile()
    return nc


def _expand_idx_v2(gidx):
    """[P, TK] -> [P, TK*8] int16 in dma_gather layout."""
    Pp, TK = gidx.shape
    wrapped = np.zeros((16, TK, 8), dtype=np.int16)
    pos = np.arange(P)
    wrapped[pos % 16, :, pos // 16] = gidx[pos, :]
    block = wrapped.transpose(0, 1, 2).reshape(16, TK * 8)
    return np.tile(block, (8, 1))


def _make_in_maps(cfg, inputs, per_core, apply_affine):
    feat_a = np.asarray(inputs["feat_a"], dtype=np.float32)
    feat_b = np.asarray(inputs["feat_b"], dtype=np.float32)
    n = feat_a.shape[0]
    fa_pad = np.zeros((cfg.npad, cfg.da), dtype=ml_dtypes.bfloat16)
    fa_pad[:n] = feat_a.astype(ml_dtypes.bfloat16)
    fb_pad = np.zeros((cfg.npad, cfg.db), dtype=ml_dtypes.bfloat16)
    fb_pad[:n] = feat_b.astype(ml_dtypes.bfloat16)

    wpa = np.ascontiguousarray(
        np.asarray(inputs["Wp_a"], dtype=np.float32)
        .reshape(cfg.fa, P, cfg.hid).transpose(1, 0, 2)).astype(ml_dtypes.bfloat16)
    wpb = np.ascontiguousarray(
        np.asarray(inputs["Wp_b"], dtype=np.float32)
        .reshape(cfg.fb, P, cfg.hid).transpose(1, 0, 2)).astype(ml_dtypes.bfloat16)
    wr = np.ascontiguousarray(
        np.asarray(inputs["Wr"], dtype=np.float32)
        .reshape(cfg.n_rel, cfg.hh, P, cfg.hid).transpose(2, 0, 1, 3)
    ).astype(ml_dtypes.bfloat16)

    in_maps = []
    for k in range(cfg.n_cores):
        sl = slice(k * cfg.shard, (k + 1) * cfg.shard)
        fta = np.ascontiguousarray(
            fa_pad[sl].T.reshape(cfg.fa, P, cfg.shard).transpose(1, 0, 2))
        ftb = np.ascontiguousarray(
            fb_pad[sl].T.reshape(cfg.fb, P, cfg.shard).transpose(1, 0, 2))
        m = {
            "featta": fta, "feattb": ftb,
            "wpa": wpa, "wpb": wpb, "wrt": wr,
            "gidx": _expand_idx_v2(per_core[k]["gidx"]),
            "dstloc": per_core[k]["dstloc"],
            "ew": per_core[k]["ew"],
            "mask": np.array([[0.0 if k == 0 else 1.0]], dtype=np.float32),
        }
        if apply_affine:
            m["gamma"] = np.broadcast_to(
                np.asarray(inputs["ln_gamma"], np.float32), (P, cfg.hid)).copy()
            m["beta"] = np.broadcast_to(
                np.asarray(inputs["ln_beta"], np.float32), (P, cfg.hid)).copy()
        in_maps.append(m)
    return in_maps


def run(cfg, inputs, sim=False):
    edge_src = np.asarray(inputs["edge_src"])
    edge_dst = np.asarray(inputs["edge_dst"])
    edge_weight = np.asarray(inputs["edge_weight"], dtype=np.float32)
    gamma = np.asarray(inputs["ln_gamma"], dtype=np.float32)
    beta = np.asarray(inputs["ln_beta"], dtype=np.float32)
    apply_affine = not (np.all(gamma == 1.0) and np.all(beta == 0.0))

    T, colbase, TK, per_core = _prep(cfg, edge_src, edge_dst, edge_weight)
    nc = _build(cfg, T, colbase, TK, apply_affine)
    in_maps = _make_in_maps(cfg, inputs, per_core, apply_affine)

    if sim:
        from concourse.bass_interp import MultiCoreSim
        msim = MultiCoreSim(nc, num_cores=cfg.n_cores)
        for k, core in enumerate(msim.cores.values()):
            for name, arr in in_maps[k].items():
                core.tensor(name)[:] = arr
        msim.simulate()
        results = [{"h1": np.array(core.tensor("h1"))}
                   for core in msim.cores.values()]
        exec_ns = None
    else:
        from concourse import bass_utils
        trace = bool(os.environ.get("BASS_KERNEL_TRACE"))
        if trace:
            _install_ntff_shim()
            bass_utils.upload_artifacts = lambda tmpdir: f"local://{tmpdir}"
        try:
            res = bass_utils.run_bass_kernel_spmd(
                nc, in_maps, core_ids=list(range(cfg.n_cores)), trace=trace)
        except Exception:
            if not trace:
                raise
            res = bass_utils.run_bass_kernel_spmd(
                nc, in_maps, core_ids=list(range(cfg.n_cores)), trace=False)
        results = res.results
        exec_ns = res.exec_time_ns
        if exec_ns is not None:
            print(f"HW exec time: {exec_ns} ns", flush=True)

    out = np.concatenate([r["h1"] for r in results], axis=0)[:cfg.n_nodes]
    return np.ascontiguousarray(out.astype(np.float32)), exec_ns


def _install_ntff_shim():
    """The image's antenv lacks axon_hooks; register the NTFF profile hook
    (ctypes over libaxon_pjrt.so) as a synthetic module so bass_utils'
    trace path finds it."""
    import types
    import ctypes
    import contextlib

    name = "antenv.axon_hooks"
    if name in sys.modules:
        return
    try:
        import antenv.axon_hooks  # noqa: F401
        return
    except ImportError:
        pass
    mod = types.ModuleType(name)
    state = {"hook": None}
    mod.set_axon_ntff_profile_hook = lambda h: state.__setitem__("hook", h)
    mod.get_axon_ntff_profile_hook = lambda: state["hook"]
    so = os.environ.get("AXON_PJRT_SO", "/opt/axon/libaxon_pjrt.so")
    if os.path.exists(so):
        try:
            lib = ctypes.CDLL(so)
            if hasattr(lib, "axon_start_nrt_profile"):
                lib.axon_start_nrt_profile.argtypes = [
                    ctypes.POINTER(ctypes.c_int64), ctypes.c_size_t]
                lib.axon_start_nrt_profile.restype = ctypes.c_int64
                lib.axon_stop_nrt_profile.argtypes = [ctypes.c_char_p]
                lib.axon_stop_nrt_profile.restype = ctypes.c_int64

                @contextlib.contextmanager
                def _hook(output_dir, device_ids):
                    import jax
                    jax.devices()
                    if device_ids:
                        ids = (ctypes.c_int64 * len(device_ids))(*device_ids)
                        rc = lib.axon_start_nrt_profile(ids, len(device_ids))
                    else:
                        rc = lib.axon_start_nrt_profile(None, 0)
                    if rc != 0:
                        raise RuntimeError(f"axon_start_nrt_profile rc={rc}")
                    try:
                        yield
                    finally:
                        nfiles = lib.axon_stop_nrt_profile(
                            str(output_dir).encode())
                        print(f"profile: {nfiles} file(s) -> {output_dir}",
                              flush=True)

                state["hook"] = _hook
        except Exception:
            pass
    sys.modules[name] = mod


def kernel(**inputs):
    cfg = Cfg()
    out, _ = run(cfg, inputs, sim=False)
    return out
